# revision 2
# baseline (speedup 1.0000x reference)
"""Trainium2 Bass kernel for nn_ConvSPE (depthwise-conv SPE + per-channel contraction).

Math (reference): per bn=(b,nu) row and channel d:
    pe_k = noise / sqrt(num*d)                       (b*num, d, s+2k)
    pe_q = depthwise_valid_xcorr(pe_k, w)            k=200 taps, same filter per channel
    qhat[b,nu,t] = sum_d pe_q[bn,d,t]      * q[b,d,t]
    khat[b,nu,t] = sum_d pe_k[bn,d,t+k//2] * k[b,d,t]

Kernel strategy (8 NeuronCores, data-parallel over the 128 bn rows; 16 rows/core):
  * d-major layouts: xt[p, d, n] = noise[row, d, 128n+p] fp16. Conv = 3
    PSUM-accumulated TensorE matmuls per 8-block output group with Toeplitz
    weights W_s[p, m] = w[p + 128s - m] (scale folded in); output PSUM is
    d-major so every downstream slice is contiguous.
  * ACT drains conv PSUM into the q-half of a combined product tile P; DVE
    multiplies in place by host-pre-arranged queries (q-part) and produces
    the khat products from xt * keys-layout (k-part).
  * The d-reduction is only taken one level on device (64 -> 32 partial
    sums); the 32 partials ship to HBM and the host does the final sum in
    fp32. Column split: cols [0:CP) reduce on GpSimd straight to fp8e4m3
    (quantization error averages down by sqrt(32) in the host sum), cols
    [CP:65) reduce on DVE in fp16. This keeps every engine near the same
    per-row busy time: PE ~2.6us, DVE ~2.8us, ACT ~2.1us, Pool ~2.6us,
    DMA ~2.6us.
  * Host decodes/sums partials, unscales, and crops khat.
"""

import math
import numpy as np

_CACHE = {}


def _ensure_paths():
    try:
        import concourse  # noqa: F401
    except ImportError:
        import sys

        for p in ("/opt/trn_rl_repo", "/root/.axon_site/_ro/trn_rl_repo"):
            if p not in sys.path:
                sys.path.insert(0, p)


N_CORES = 8
B, D, L, K, NUM = 4, 64, 4096, 200, 32
NW = 34  # x blocks of 128 per row (conv reads blocks tau+s, tau<32, s<3)
NT = 32  # qhat output blocks
NK = 33  # khat product blocks (u = t + 100 spans [0, 4224))
NJ = NT + NK  # combined product columns (q | k)
ROWS = 16  # bn rows per core
CP = 40  # cols [0:CP) reduce on GpSimd -> fp8; [CP:NJ) on DVE -> fp16
QS = 8.0  # extra scale folded into queries (fp8 partial headroom)
KS = 16.0  # extra scale folded into keys


def build_module():
    """Build + compile the per-core Bass module (identical SPMD program)."""
    _ensure_paths()
    from contextlib import ExitStack

    import concourse.bacc as bacc
    import concourse.mybir as mybir
    import concourse.tile as tile

    F16 = mybir.dt.float16
    F32 = mybir.dt.float32
    F8 = mybir.dt.float8e4

    nc = bacc.Bacc(
        "TRN2", target_bir_lowering=False, debug=False, num_devices=N_CORES
    )

    xf_d = nc.dram_tensor("xf", [128, ROWS, D, NW], F16, kind="ExternalInput").ap()
    wq_d = nc.dram_tensor("wq", [3, 128, 128], F16, kind="ExternalInput").ap()
    qk_d = nc.dram_tensor("qk", [128, D, NJ], F16, kind="ExternalInput").ap()
    p8_d = nc.dram_tensor("p8", [128, ROWS, 32, CP], F8, kind="ExternalOutput").ap()
    p16_d = nc.dram_tensor(
        "p16", [128, ROWS, 32, NJ - CP], F16, kind="ExternalOutput"
    ).ap()

    with tile.TileContext(nc) as tc, ExitStack() as ctx:
        wp = ctx.enter_context(tc.tile_pool(name="const", bufs=1))
        xp = ctx.enter_context(tc.tile_pool(name="x", bufs=4))
        pp = ctx.enter_context(tc.tile_pool(name="psum", bufs=3, space="PSUM"))
        cp_ = ctx.enter_context(tc.tile_pool(name="prod", bufs=3))
        o8p = ctx.enter_context(tc.tile_pool(name="o8", bufs=3))
        o16p = ctx.enter_context(tc.tile_pool(name="o16", bufs=3))

        wall = wp.tile([128, 3, 128], F16, tag="wall")
        nc.sync.dma_start(wall[:], wq_d.transpose([1, 0, 2]))
        wts = [wall[:, s, :] for s in range(3)]
        qk_t = wp.tile([128, D, NJ], F16, tag="qk")
        nc.scalar.dma_start(qk_t[:], qk_d)

        def emit_row(r):
            xt = xp.tile([128, D, NW], F16, tag="xt", name=f"xt_{r}")
            if r % 2 == 0:
                nc.sync.dma_start(xt[:], xf_d[:, r])
            else:
                nc.scalar.dma_start(xt[:], xf_d[:, r])

            P = cp_.tile([128, D, NJ], F16, tag="P", name=f"P_{r}")
            # conv: 2 PSUM halves, 3 Toeplitz bands x 2 8-block groups each
            for h in range(2):
                ps = pp.tile([128, D, 16], F32, tag="ps", name=f"ps_{r}_{h}")
                for s in range(3):
                    for g in range(2 * h, 2 * h + 2):
                        nc.tensor.matmul(
                            ps[:, :, (g - 2 * h) * 8 : (g - 2 * h + 1) * 8],
                            wts[s],
                            xt[:, :, g * 8 + s : g * 8 + s + 8],
                            start=(s == 0),
                            stop=(s == 2),
                        )
                nc.scalar.copy(P[:, :, 16 * h : 16 * (h + 1)], ps[:])

            # products: q-part in place over the drained conv, k-part from xt
            nc.vector.tensor_mul(P[:, :, 0:NT], P[:, :, 0:NT], qk_t[:, :, 0:NT])
            nc.vector.tensor_mul(P[:, :, NT:NJ], xt[:, :, 0:NK], qk_t[:, :, NT:NJ])

            # one reduction level (64 -> 32 partials), split Pool/DVE
            p8 = o8p.tile([128, 32, CP], F8, tag="p8", name=f"p8_{r}")
            p16 = o16p.tile([128, 32, NJ - CP], F16, tag="p16", name=f"p16_{r}")
            nc.gpsimd.tensor_add(p8[:], P[:, 0:32, 0:CP], P[:, 32:64, 0:CP])
            nc.vector.tensor_add(p16[:], P[:, 0:32, CP:NJ], P[:, 32:64, CP:NJ])
            nc.sync.dma_start(p8_d[:, r], p8[:])
            nc.scalar.dma_start(p16_d[:, r], p16[:])

        for r in range(ROWS):
            emit_row(r)

    nc.compile()
    return nc


def _get_module():
    if "nc" not in _CACHE:
        _CACHE["nc"] = build_module()
    return _CACHE["nc"]


def make_in_maps(queries, keys, noise, conv_weight, num):
    """Host-side shard + re-layout (all cheap numpy ops)."""
    num = int(np.asarray(num))
    queries = np.asarray(queries, dtype=np.float32)
    keys = np.asarray(keys, dtype=np.float32)
    noise = np.asarray(noise, dtype=np.float32)
    w = np.asarray(conv_weight, dtype=np.float32)[0, 0, :]
    scale = 1.0 / math.sqrt(num * D)

    # Toeplitz weights (scale folded in): W_s[p, m] = w[p + 128s - m] * scale
    p = np.arange(128)[:, None]
    m = np.arange(128)[None, :]
    Wq = np.zeros((3, 128, 128), np.float32)
    for s in range(3):
        j = p + 128 * s - m
        mask = (j >= 0) & (j < K)
        Wq[s][mask] = w[j[mask]] * scale
    Wq16 = Wq.astype(np.float16)

    # xf[core][p, row, d, n] = noise[16c+row, d, 128n + p]  (d-major)
    xf = (
        noise[:, :, : NW * 128]
        .reshape(B * NUM, D, NW, 128)
        .transpose(3, 0, 1, 2)
        .astype(np.float16)
    )  # [128, B*NUM, D, NW]
    # qk[b][p, d, 0:NT]  = queries[b, d, 128 tau + p] * QS
    # qk[b][p, d, NT:NJ] = keys[b, d, 128n + p - 100] * scale * KS (zero OOB)
    qt = queries.reshape(B, D, NT, 128).transpose(0, 3, 1, 2) * QS
    kp = np.zeros((B, D, NK * 128), np.float32)
    kp[:, :, K // 2 : K // 2 + L] = keys * (scale * KS)
    kf = kp.reshape(B, D, NK, 128).transpose(0, 3, 1, 2)
    qk = np.concatenate([qt, kf], axis=3).astype(np.float16)  # [128, D, NJ]

    in_maps = []
    for c in range(N_CORES):
        b = c // 2
        in_maps.append(
            {
                "xf": np.ascontiguousarray(xf[:, ROWS * c : ROWS * (c + 1)]),
                "wq": Wq16,
                "qk": np.ascontiguousarray(qk[b]),
            }
        )
    return in_maps


def assemble_outputs(results):
    qhat = np.empty((B * NUM, L), np.float32)
    khat = np.empty((B * NUM, L), np.float32)
    for c in range(N_CORES):
        p8 = np.asarray(results[c]["p8"]).astype(np.float32)  # [128, R, 32, CP]
        p16 = np.asarray(results[c]["p16"]).astype(np.float32)  # [128, R, 32, NJ-CP]
        part = np.concatenate([p8, p16], axis=3)  # [128, R, 32, NJ]
        red = part.sum(axis=2)  # [128, R, NJ]
        qv = red[:, :, 0:NT] / QS  # [128, R, NT]
        kv = red[:, :, NT:NJ] / KS  # [128, R, NK]
        qhat[ROWS * c : ROWS * (c + 1)] = qv.transpose(1, 2, 0).reshape(ROWS, L)
        kfull = kv.transpose(1, 2, 0).reshape(ROWS, NK * 128)
        khat[ROWS * c : ROWS * (c + 1)] = kfull[:, K // 2 : K // 2 + L]
    return (
        qhat.reshape(B, NUM, L),
        khat.reshape(B, NUM, L),
    )


def kernel(queries, keys, noise, conv_weight, num):
    _ensure_paths()
    from concourse import bass_utils

    in_maps = make_in_maps(queries, keys, noise, conv_weight, num)
    nc = _get_module()
    res = bass_utils.run_bass_kernel_spmd(nc, in_maps, core_ids=list(range(N_CORES)))
    return assemble_outputs(res.results)


# revision 23
# speedup vs baseline: 1.5253x; 1.5253x over previous
"""Trainium2 Bass kernel for nn_ConvSPE (depthwise-conv SPE + per-channel contraction).

Math (reference): per bn=(b,nu) row and channel d:
    pe_k = noise / sqrt(num*d)                       (b*num, d, s+2k)
    pe_q = depthwise_valid_xcorr(pe_k, w)            k=200 taps, same filter per channel
    qhat[b,nu,t] = sum_d pe_q[bn,d,t]      * q[b,d,t]
    khat[b,nu,t] = sum_d pe_k[bn,d,t+k//2] * k[b,d,t]

Kernel strategy (8 NeuronCores, data-parallel over the 128 bn rows; 16 rows/core):
  * d-major layouts: xt[p, d, n] = noise[row, d, 128n+p] fp16. Conv = 3
    PSUM-accumulated TensorE matmuls per 8-block output group with Toeplitz
    weights W_s[p, m] = w[p + 128s - m] (scale folded in). Each 8-block
    group owns one full PSUM bank (accumulation groups must not share a
    2KB zero-region). PSUM is d-major so downstream slices are contiguous.
  * ACT drains conv PSUM into the q-half of a combined product tile P
    [128, d=64, j=65] (j = 32 qhat tau-blocks | 33 khat u-blocks); DVE
    multiplies in place by host-pre-arranged queries (q-part) and forms
    khat products from xt * keys-layout (k-part, shift folded on host).
  * The d-reduction only goes one or two levels on device; the host sums
    the shipped fp16 partials in fp32. Column routing balances DVE / Pool
    / DMA time: cols [0:N16) get DVE L1+L2 and ship 16 partials; cols
    [N16:65) get GpSimd L1 (k cols first; last row on DVE to shorten the
    tail) and ship 32 partials.
  * Pipelined emission: reduce lags conv by one row, ships lag two; all
    ships ride one contiguous per-row DMA. PE warmup matmuls at t~0 (from
    a memset scratch) carry the TensorE p-state ramp so every conv matmul
    runs at full clock.
  * Everything stays fp16 on device; host does final sums/crops in fp32.
"""

import math
import numpy as np

_CACHE = {}


def _ensure_paths():
    try:
        import concourse  # noqa: F401
    except ImportError:
        import sys

        for p in ("/opt/trn_rl_repo", "/root/.axon_site/_ro/trn_rl_repo"):
            if p not in sys.path:
                sys.path.insert(0, p)


N_CORES = 8
B, D, L, K, NUM = 4, 64, 4096, 200, 32
NW = 34  # x blocks of 128 per row (conv reads blocks tau+s, tau<32, s<3)
NT = 32  # qhat output blocks
NK = 33  # khat product blocks (u = t + 100 spans [0, 4224))
NJ = NT + NK  # combined product columns (q | k)
ROWS = 16  # bn rows per core

# Column routing (tunable): [0:NPE) on PE, [NPE:NPE+N16) ship-16 via DVE
# L1+L2, [NPE+N16:NJ) ship-32 with L1 split Pool/DVE by N32DVE.
NPE = 0
N16 = 22
N32 = NJ - NPE - N16  # 29
N32DVE = 0  # leading cols of the ship-32 range whose L1 runs on DVE
NWARM = 18  # PE warmup matmuls
NTAILDVE = 1  # trailing rows whose ship-32 L1 runs on DVE instead of Pool
NRAW = 0  # trailing rows that ship raw 64-wide products (no on-device L1/L2)


def build_module():
    """Build + compile the per-core Bass module (identical SPMD program)."""
    _ensure_paths()
    from contextlib import ExitStack

    import concourse.bacc as bacc
    import concourse.mybir as mybir
    import concourse.tile as tile

    F16 = mybir.dt.float16
    F32 = mybir.dt.float32

    nc = bacc.Bacc(
        "TRN2", target_bir_lowering=False, debug=False, num_devices=N_CORES
    )

    c16 = NPE + N16  # start of ship-32 range

    xf_d = nc.dram_tensor("xf", [128, ROWS, D, NW], F16, kind="ExternalInput").ap()
    wq_d = nc.dram_tensor("wq", [128, 4, 128], F16, kind="ExternalInput").ap()
    qq_d = nc.dram_tensor("qq", [128, D, NT], F16, kind="ExternalInput").ap()
    qkk_d = nc.dram_tensor("qkk", [128, D, NK], F16, kind="ExternalInput").ap()
    XOUT = 16 * N16 + 32 * N32
    out_d = nc.dram_tensor("out", [128, ROWS, XOUT], F16, kind="ExternalOutput").ap()
    raw_d = (
        nc.dram_tensor(
            "raw", [128, NRAW, D, NJ], F16, kind="ExternalOutput"
        ).ap()
        if NRAW > 0
        else None
    )

    with tile.TileContext(nc) as tc, ExitStack() as ctx:
        wp = ctx.enter_context(tc.tile_pool(name="const", bufs=1))
        xp = ctx.enter_context(tc.tile_pool(name="x", bufs=5))
        pp = ctx.enter_context(tc.tile_pool(name="psum", bufs=7, space="PSUM"))
        rp = ctx.enter_context(tc.tile_pool(name="rpsum", bufs=1, space="PSUM"))
        cp_ = ctx.enter_context(tc.tile_pool(name="prod", bufs=4))
        op_ = ctx.enter_context(tc.tile_pool(name="out", bufs=4))

        # Toeplitz bands + identity in one const DMA
        wall = wp.tile([128, 4, 128], F16, tag="wall")
        nc.sync.dma_start(wall[:], wq_d)
        wts = [wall[:, s, :] for s in range(3)]
        ident = wall[:, 3, :]
        qq_t = wp.tile([128, D, NT], F16, tag="qq")
        qkk_t = wp.tile([128, D, NK], F16, tag="qkk")

        def emit_load(r):
            xt = xp.tile([128, D, NW], F16, tag="xt", name=f"xt_{r}")
            nc.sync.dma_start(xt[:], xf_d[:, r])
            return xt

        def emit_conv(r, xt):
            P = cp_.tile([128, D, NJ], F16, tag="P", name=f"P_{r}")
            # conv: 4 bank-sized PSUM groups (accumulation group = 1 bank),
            # 3 Toeplitz bands each
            for g in range(4):
                ps = pp.tile([128, D, 8], F32, tag="ps", name=f"ps_{r}_{g}")
                for s in range(3):
                    nc.tensor.matmul(
                        ps[:],
                        wts[s],
                        xt[:, :, g * 8 + s : g * 8 + s + 8],
                        start=(s == 0),
                        stop=(s == 2),
                    )
                nc.scalar.copy(P[:, :, g * 8 : (g + 1) * 8], ps[:])
            return P

        def emit_muls(r, xt, P):
            # products: k-part from xt (ready first), q-part in place on conv
            nc.vector.tensor_mul(P[:, :, NT:NJ], xt[:, :, 0:NK], qkk_t[:])
            nc.vector.tensor_mul(P[:, :, 0:NT], P[:, :, 0:NT], qq_t[:])

        def emit_reduce(r, xt, P):
            OUT = op_.tile([128, XOUT], F16, tag="OUT", name=f"OUT_{r}")
            o_16 = OUT[:, 0 : 16 * N16].rearrange("p (a b) -> p a b", a=16, b=N16)
            o_32 = OUT[:, 16 * N16 : XOUT].rearrange(
                "p (a b) -> p a b", a=32, b=N32
            )

            emit_muls(r, xt, P)

            # cols [0:N16): DVE L1+L2, ship 16
            nc.vector.tensor_add(
                P[:, 0:32, 0:N16], P[:, 0:32, 0:N16], P[:, 32:64, 0:N16]
            )
            nc.vector.tensor_add(o_16, P[:, 0:16, 0:N16], P[:, 16:32, 0:N16])

            # cols [N16:NJ): L1 on GpSimd (k cols first: their products are
            # ready before the q-part drain+mul completes); the trailing rows
            # run it on DVE so the pipeline tail is short
            if r >= ROWS - NTAILDVE:
                nc.vector.tensor_add(
                    o_32, P[:, 0:32, N16:NJ], P[:, 32:64, N16:NJ]
                )
            else:
                nc.gpsimd.tensor_add(
                    o_32[:, :, NT - N16 : N32],
                    P[:, 0:32, NT:NJ],
                    P[:, 32:64, NT:NJ],
                )
                nc.gpsimd.tensor_add(
                    o_32[:, :, 0 : NT - N16],
                    P[:, 0:32, N16:NT],
                    P[:, 32:64, N16:NT],
                )
            return OUT

        def emit_ship(r, OUT):
            nc.sync.dma_start(out_d[:, r], OUT[:])

        # PE warmup: keep the PE busy from t~1us so the p-state ramp is
        # complete when the first conv lands (cost model: 3us continuous
        # busy => full clock).
        wsc = wp.tile([128, 2, 128], F16, tag="wsc")
        nc.vector.memset(wsc[:], 0.0)
        wps = rp.tile([128, 256], F32, tag="wps", name="wps")
        for i in range(NWARM):
            nc.tensor.matmul(
                wps[:],
                wsc[:, 0, :],
                wsc[:],
                start=(i == 0),
                stop=(i == NWARM - 1),
            )

        NREG = ROWS - NRAW
        xts, Ps, OUTs = {}, {}, {}
        for r in range(ROWS):
            xts[r] = emit_load(r)
            if r == 0:
                nc.sync.dma_start(qkk_t[:], qkk_d)
            elif r == 1:
                nc.sync.dma_start(qq_t[:], qq_d)
            Ps[r] = emit_conv(r, xts[r])
            if r >= 1:
                rr = r - 1
                if rr < NREG:
                    OUTs[rr] = emit_reduce(rr, xts[rr], Ps[rr])
                else:
                    emit_muls(rr, xts[rr], Ps[rr])
                    nc.sync.dma_start(raw_d[:, rr - NREG], Ps[rr][:])
            if r >= 2 and r - 2 < NREG:
                emit_ship(r - 2, OUTs[r - 2])
        if NRAW > 0:
            emit_ship(NREG - 1, OUTs[NREG - 1])
            emit_muls(ROWS - 1, xts[ROWS - 1], Ps[ROWS - 1])
            nc.sync.dma_start(raw_d[:, NRAW - 1], Ps[ROWS - 1][:])
        else:
            OUTs[ROWS - 1] = emit_reduce(
                ROWS - 1, xts[ROWS - 1], Ps[ROWS - 1]
            )
            emit_ship(ROWS - 2, OUTs[ROWS - 2])
            emit_ship(ROWS - 1, OUTs[ROWS - 1])

    nc.compile()
    return nc


def _get_module():
    if "nc" not in _CACHE:
        _CACHE["nc"] = build_module()
    return _CACHE["nc"]


def make_in_maps(queries, keys, noise, conv_weight, num):
    """Host-side shard + re-layout (all cheap numpy ops)."""
    num = int(np.asarray(num))
    queries = np.asarray(queries, dtype=np.float32)
    keys = np.asarray(keys, dtype=np.float32)
    noise = np.asarray(noise, dtype=np.float32)
    w = np.asarray(conv_weight, dtype=np.float32)[0, 0, :]
    scale = 1.0 / math.sqrt(num * D)

    # Toeplitz weights (scale folded in): W_s[p, m] = w[p + 128s - m] * scale
    p = np.arange(128)[:, None]
    m = np.arange(128)[None, :]
    Wq = np.zeros((4, 128, 128), np.float32)
    for s in range(3):
        j = p + 128 * s - m
        mask = (j >= 0) & (j < K)
        Wq[s][mask] = w[j[mask]] * scale
    Wq[3] = np.eye(128, dtype=np.float32)
    Wq16 = np.ascontiguousarray(Wq.astype(np.float16).transpose(1, 0, 2))

    # xf[core][p, row, d, n] = noise[16c+row, d, 128n + p]  (d-major)
    xf = (
        noise[:, :, : NW * 128]
        .reshape(B * NUM, D, NW, 128)
        .transpose(3, 0, 1, 2)
        .astype(np.float16)
    )  # [128, B*NUM, D, NW]
    # qk[b][p, d, 0:NT]  = queries[b, d, 128 tau + p]
    # qk[b][p, d, NT:NJ] = keys[b, d, 128n + p - 100] * scale (zero OOB)
    qt = queries.reshape(B, D, NT, 128).transpose(0, 3, 1, 2)
    kp = np.zeros((B, D, NK * 128), np.float32)
    kp[:, :, K // 2 : K // 2 + L] = keys * scale
    kf = kp.reshape(B, D, NK, 128).transpose(0, 3, 1, 2)
    qq = np.ascontiguousarray(qt.astype(np.float16))  # [B, 128, D, NT]
    qkk = np.ascontiguousarray(kf.astype(np.float16))  # [B, 128, D, NK]

    in_maps = []
    for c in range(N_CORES):
        b = c // 2
        in_maps.append(
            {
                "xf": np.ascontiguousarray(xf[:, ROWS * c : ROWS * (c + 1)]),
                "wq": Wq16,
                "qq": qq[b],
                "qkk": qkk[b],
            }
        )
    return in_maps


def assemble_outputs(results):
    c16 = NPE + N16
    qhat = np.empty((B * NUM, L), np.float32)
    khat = np.empty((B * NUM, L), np.float32)
    for c in range(N_CORES):
        out = np.asarray(results[c]["out"]).astype(np.float32)  # [128, R, XOUT]
        raw = (
            np.asarray(results[c]["raw"]).astype(np.float32)
            if NRAW > 0
            else None
        )
        XOUT = 16 * N16 + 32 * N32
        NREG = ROWS - NRAW
        o16 = out[:, :, 0 : 16 * N16].reshape(128, ROWS, 16, N16)
        o32 = out[:, :, 16 * N16 : XOUT].reshape(128, ROWS, 32, N32)
        red = np.empty((128, ROWS, NJ), np.float32)
        red[:, :NREG, 0:N16] = o16[:, :NREG].sum(axis=2)
        red[:, :NREG, N16:NJ] = o32[:, :NREG].sum(axis=2)
        if NRAW > 0:
            red[:, NREG:] = raw.sum(axis=2)
        qv = red[:, :, 0:NT]  # [128, R, NT]
        kv = red[:, :, NT:NJ]  # [128, R, NK]
        qhat[ROWS * c : ROWS * (c + 1)] = qv.transpose(1, 2, 0).reshape(ROWS, L)
        kfull = kv.transpose(1, 2, 0).reshape(ROWS, NK * 128)
        khat[ROWS * c : ROWS * (c + 1)] = kfull[:, K // 2 : K // 2 + L]
    return (
        qhat.reshape(B, NUM, L),
        khat.reshape(B, NUM, L),
    )


def kernel(queries, keys, noise, conv_weight, num):
    _ensure_paths()
    from concourse import bass_utils

    in_maps = make_in_maps(queries, keys, noise, conv_weight, num)
    nc = _get_module()
    res = bass_utils.run_bass_kernel_spmd(nc, in_maps, core_ids=list(range(N_CORES)))
    return assemble_outputs(res.results)
